# revision 20
# baseline (speedup 1.0000x reference)
"""AttentionBlock (GroupNorm + single-head self-attention + proj + residual)
on 8 Trainium2 NeuronCores.

Sharding: batch (4) x query-token-half (2) -> 8 shards. Each core gets the
full image of its batch element (for GroupNorm stats and K/V over all 4096
tokens) plus its half of the query tokens; K/V/GN are computed redundantly
by the 2 cores sharing a batch element, which is far cheaper than
cross-core collectives at this size.

Math per core (c=256 channels, n=4096 tokens, nq=2048 query tokens):
  GroupNorm is folded into the QKV weights: xn = s_c * x + t_c with
  per-channel s,t computed on-device from group stats, so
  Q = (wq*s) @ x + (wq@t + bq), etc. The score scale 1/sqrt(c) is folded
  into wk/bk on the host. The V-path bias is folded into the output
  projection bias (b* = wp@(wv@t+bv) + bp).
  Scores are computed k-major: S^T[m,i] = sum_o K[o,m] Q[o,i] so softmax
  needs a cross-partition denominator, obtained by accumulating exp tiles
  on DVE and one all-ones matmul (which also broadcasts the sums to all
  partitions); A@V uses lhsT = V^T (computed directly as x^T @ wv') so no
  transposes are needed anywhere. The attention loop processes all 1024
  query columns of a half-shard at once (2-bank PSUM tiles) to halve the
  ACT/DVE instruction count.

All matmuls run in float32r (TF32-like: fp32 with 11-bit mantissa, full
fp32 accumulate) which streams near bf16 rate -- measured ~2e-4 relative
error on the full block, ~15x better than bf16.
"""

import numpy as np

B, C, H, W = 4, 256, 64, 64
N = H * W            # 4096 tokens
NQ = N // 2          # 2048 query tokens per core
GROUPS = 8
GSIZE = C // GROUPS  # 32 channels per group
EPS = 1e-5
P = 128              # partitions
CC = C // P          # 2 channel chunks
NCORES = 8
QW = 1024            # query columns processed per attention pass
ATTN_BF16 = False    # K/Q/V^T/exp tiles in bf16 (S^T and A@V matmuls run at
                     # bf16 stream rate; scores still accumulate in fp32)
NQP = NQ // QW       # 2 passes

_cache = {}


def round_tf32(x: np.ndarray) -> np.ndarray:
    """Round fp32 to fp32r (11-bit mantissa, round-to-nearest-even)."""
    i = np.ascontiguousarray(x, dtype=np.float32).view(np.uint32)
    r = (i + np.uint32(0x7FF) + ((i >> np.uint32(12)) & np.uint32(1))) & np.uint32(0xFFFFF000)
    return r.view(np.float32)


def build_nc():
    import concourse.bass as bass
    import concourse.mybir as mybir
    import concourse.tile as tile
    from concourse import bacc

    F32 = mybir.dt.float32
    F32R = mybir.dt.float32r
    BF16 = mybir.dt.bfloat16
    ADT = BF16 if ATTN_BF16 else F32R
    AF = mybir.ActivationFunctionType
    OP = mybir.AluOpType

    nc = bacc.Bacc(None, target_bir_lowering=False)

    # ---------- I/O ----------
    x_d = nc.dram_tensor("x_r", [C, N], F32R, kind="ExternalInput")
    # packed weights [C, 4*C]: wq_t | wk_t | wv_t | wp_t, and packed
    # per-channel vectors [C, 6]: bq bk bv bp gamma beta
    wall_d = nc.dram_tensor("w_all", [C, 4 * C], F32, kind="ExternalInput")
    vall_d = nc.dram_tensor("v_all", [C, 6], F32, kind="ExternalInput")
    y_d = nc.dram_tensor("y", [C, NQ], F32, kind="ExternalOutput")

    # constants (fp32r via bitcast; all values exactly representable)
    ind1_np = np.zeros((P, 4), dtype=np.float32)
    for c in range(P):
        ind1_np[c, c // GSIZE] = 1.0 / GSIZE
    ind2_np = np.zeros((4, P), dtype=np.float32)
    for c in range(P):
        ind2_np[c // GSIZE, c] = 1.0
    cpack_np = np.concatenate([ind1_np, np.ones((P, P), np.float32)], axis=1)
    cpack_d = nc.inline_tensor(cpack_np, name="cpack").bitcast(F32R)
    ind2_d = nc.inline_tensor(ind2_np, name="ind2").bitcast(F32R)

    NI = N // P        # 32 key-token chunks

    with tile.TileContext(nc) as tc:
        with tc.tile_pool(name="persist", bufs=1) as pp, \
             tc.tile_pool(name="small", bufs=2) as sp, \
             tc.tile_pool(name="work", bufs=2) as wkp, \
             tc.tile_pool(name="etp", bufs=3) as etp, \
             tc.tile_pool(name="psA", bufs=2, space="PSUM") as psA, \
             tc.tile_pool(name="psB", bufs=2, space="PSUM") as psB:

            # ---------- load (DMA-instruction count kept minimal: each
            # dma_start costs ~600ns of queue issue time) ----------
            # consts + vectors first (they gate the GroupNorm matmuls)
            cpack_s = pp.tile([P, 4 + P], F32R, name="cpack_s")
            nc.sync.dma_start(out=cpack_s, in_=cpack_d[:, :])
            ind1_s = cpack_s[:, 0:4]
            allones_s = cpack_s[:, 4:4 + P]
            ind2_s = pp.tile([4, P], F32R, name="ind2s")
            nc.scalar.dma_start(out=ind2_s, in_=ind2_d[:, :])
            vall = []
            for cc in range(CC):
                t = pp.tile([P, 6], F32, name=f"vall{cc}")
                nc.scalar.dma_start(out=t, in_=vall_d[cc * P:(cc + 1) * P, :])
                vall.append(t)
            vecs = {}
            for vi, nm in enumerate(("bq", "bk", "bv", "bp", "gam", "bet")):
                for cc in range(CC):
                    vecs[(nm, cc)] = vall[cc][:, vi:vi + 1]
            eps4 = pp.tile([4, 1], F32, name="eps4")
            nc.vector.memset(eps4, EPS)
            # x in 1024-col chunks, interleaved across the two HWDGE queues
            xs = []
            for cc in range(CC):
                t = pp.tile([P, N], F32R, name=f"xs{cc}")
                xs.append(t)
            for j in range(4):
                for cc in range(CC):
                    eng = nc.sync if cc == 0 else nc.scalar
                    eng.dma_start(out=xs[cc][:, j * 1024:(j + 1) * 1024],
                                  in_=x_d[cc * P:(cc + 1) * P, j * 1024:(j + 1) * 1024])
            wraw = {}
            for cc in range(CC):
                t = pp.tile([P, 4 * C], F32, name=f"wall{cc}")
                nc.scalar.dma_start(out=t, in_=wall_d[cc * P:(cc + 1) * P, :])
                for wi, nm in enumerate(("wq", "wk", "wv", "wp")):
                    wraw[(nm, cc)] = t[:, wi * C:(wi + 1) * C]



            # ---------- GroupNorm stats -> per-channel scale/shift ----------
            s_vecs = []   # [128,1] f32 per cc: s_c = rstd_g * gamma_c
            t_vecs = []   # [128,1] f32 per cc: t_c = beta_c - mean_g * s_c
            statst = [sp.tile([P, 8, 6], F32, name=f"bnstats{cc}", tag=f"bnstats{cc}")
                      for cc in range(CC)]
            for cc in range(CC):
                for sg in range(8):
                    nc.vector.bn_stats(out=statst[cc][:, sg, :],
                                       in_=xs[cc].bitcast(F32)[:, sg * 512:(sg + 1) * 512])
            for cc in range(CC):
                eng = nc.vector
                xf = xs[cc].bitcast(F32)
                st2 = sp.tile([P, 2], F32R, name="gnst2")
                mv = sp.tile([P, 2], F32, name="bnmv")
                eng.bn_aggr(out=mv, in_=statst[cc])
                # st2 = (mean, E[x^2]) per channel, as fp32r
                m2 = sp.tile([P, 1], F32, name="gnm2")
                eng.tensor_mul(out=m2, in0=mv[:, 0:1], in1=mv[:, 0:1])
                eng.tensor_copy(out=st2[:, 0:1], in_=mv[:, 0:1])
                eng.tensor_tensor(out=st2[:, 1:2], in0=mv[:, 1:2], in1=m2, op=OP.add)
                # group means of (mean, E[x^2]) via indicator matmul
                pg = psB.tile([4, 2], F32, name="psg", tag="pav")
                nc.tensor.matmul(pg, ind1_s, st2, start=True, stop=True)
                pgs = sp.tile([4, 2], F32, name="gnpgs")
                eng.tensor_copy(out=pgs, in_=pg)
                gm2 = sp.tile([4, 1], F32, name="gngm2")
                eng.tensor_mul(out=gm2, in0=pgs[:, 0:1], in1=pgs[:, 0:1])
                gvar = sp.tile([4, 1], F32, name="gnvar")
                eng.tensor_tensor(out=gvar, in0=pgs[:, 1:2], in1=gm2, op=OP.subtract)
                gstd = sp.tile([4, 1], F32, name="gnstd")
                nc.scalar.activation(out=gstd, in_=gvar, func=AF.Sqrt, bias=eps4, scale=1.0)
                grstd = sp.tile([4, 1], F32, name="gnrstd")
                nc.vector.reciprocal(out=grstd, in_=gstd)
                gvals = sp.tile([4, 2], F32R, name="gnvals")
                eng.tensor_copy(out=gvals[:, 0:1], in_=pgs[:, 0:1])
                eng.tensor_copy(out=gvals[:, 1:2], in_=grstd)
                # broadcast group (mean, rstd) back to channels
                pb = psB.tile([P, 2], F32, name="psb2", tag="pav")
                nc.tensor.matmul(pb, ind2_s, gvals, start=True, stop=True)
                s_v = sp.tile([P, 1], F32, name="gns")
                eng.tensor_mul(out=s_v, in0=pb[:, 1:2], in1=vecs[("gam", cc)])
                ms = sp.tile([P, 1], F32, name="gnms")
                eng.tensor_mul(out=ms, in0=pb[:, 0:1], in1=s_v)
                t_v = sp.tile([P, 1], F32, name="gnt")
                eng.tensor_tensor(out=t_v, in0=vecs[("bet", cc)], in1=ms, op=OP.subtract)
                s_vecs.append(s_v)
                t_vecs.append(t_v)

            # ---------- fold GN into weights; effective biases ----------
            wr = {}
            for nm in ("wq", "wk", "wv"):
                for cc in range(CC):
                    t = pp.tile([P, C], F32R, name=f"{nm}r{cc}")
                    nc.vector.tensor_scalar_mul(out=t, in0=wraw[(nm, cc)], scalar1=s_vecs[cc])
                    wr[(nm, cc)] = t
            for cc in range(CC):
                t = pp.tile([P, C], F32R, name=f"wpr{cc}")
                nc.vector.tensor_copy(out=t, in_=wraw[("wp", cc)])
                wr[("wp", cc)] = t

            beff = {}
            for nm in ("wq", "wk", "wv"):
                bnm = "b" + nm[1]
                for oc in range(CC):
                    pbx = psB.tile([P, 1], F32, name="psbias", tag="pav")
                    for cc in range(CC):
                        # raw (unfolded) weights: bias is w @ t, not (w*s) @ t.
                        # fp32 matmul is fine here (N=1).
                        nc.tensor.matmul(pbx, wraw[(nm, cc)][:, oc * P:(oc + 1) * P],
                                         t_vecs[cc], start=(cc == 0), stop=(cc == CC - 1))
                    t = pp.tile([P, 1], F32, name=f"beff_{nm}{oc}")
                    nc.scalar.activation(out=t, in_=pbx, func=AF.Identity,
                                         bias=vecs[(bnm, oc)], scale=1.0)
                    beff[(nm, oc)] = t
            # b* = wp @ bv_eff + bp (V bias folded through the projection)
            for oc in range(CC):
                pbx = psB.tile([P, 1], F32, name="psbias2", tag="pav")
                for cc in range(CC):
                    nc.tensor.matmul(pbx, wraw[("wp", cc)][:, oc * P:(oc + 1) * P],
                                     beff[("wv", cc)], start=(cc == 0), stop=(cc == CC - 1))
                t = pp.tile([P, 1], F32, name=f"bstar{oc}")
                nc.scalar.activation(out=t, in_=pbx, func=AF.Identity,
                                     bias=vecs[("bp", oc)], scale=1.0)
                beff[("wp", oc)] = t

            # ---------- projections ----------
            Qs = [pp.tile([P, NQ], ADT, name=f"Q{oc}") for oc in range(CC)]
            Ks = [pp.tile([P, N], ADT, name=f"K{oc}") for oc in range(CC)]
            for oc in range(CC):
                for i in range(NQ // QW):
                    pq = psA.tile([P, QW], F32, name="psq", tag="pst")
                    for h in range(2):
                        sl = slice(i * QW + h * 512, i * QW + (h + 1) * 512)
                        for cc in range(CC):
                            nc.tensor.matmul(pq[:, h * 512:(h + 1) * 512],
                                             wr[("wq", cc)][:, oc * P:(oc + 1) * P],
                                             xs[cc][:, sl],
                                             start=(cc == 0), stop=(cc == CC - 1))
                    nc.scalar.activation(out=Qs[oc][:, i * QW:(i + 1) * QW], in_=pq,
                                         func=AF.Identity, bias=beff[("wq", oc)], scale=1.0)
                for i in range(N // QW):
                    pk = psA.tile([P, QW], F32, name="psk", tag="pst")
                    for h in range(2):
                        sl = slice(i * QW + h * 512, i * QW + (h + 1) * 512)
                        for cc in range(CC):
                            nc.tensor.matmul(pk[:, h * 512:(h + 1) * 512],
                                             wr[("wk", cc)][:, oc * P:(oc + 1) * P],
                                             xs[cc][:, sl],
                                             start=(cc == 0), stop=(cc == CC - 1))
                    nc.scalar.activation(out=Ks[oc][:, i * QW:(i + 1) * QW], in_=pk,
                                         func=AF.Identity, bias=beff[("wk", oc)], scale=1.0)
            VTs = pp.tile([P, NI * C], ADT, name="VTs")  # [128 tok, 32*256]
            for it in range(0, NI, 2):
                pv = psA.tile([P, 512], F32, name="psv", tag="pst")
                for j in range(2):
                    for cc in range(CC):
                        nc.tensor.matmul(pv[:, j * C:(j + 1) * C],
                                         xs[cc][:, (it + j) * P:(it + j + 1) * P],
                                         wr[("wv", cc)],
                                         start=(cc == 0), stop=(cc == CC - 1))
                nc.vector.tensor_copy(out=VTs[:, it * C:(it + 2) * C], in_=pv)

            # ---------- attention (QW=1024 query columns per pass) ----------
            for qp in range(NQP):
                pav = [psB.tile([P, QW], F32, name=f"pav{cc}", tag="pav") for cc in range(CC)]
                acc = etp.tile([P, QW], F32R, name="acc", tag="acc")
                accf = acc.bitcast(F32)
                acc_engs = [nc.vector, nc.gpsimd]
                for m in range(NI):
                    pst = psA.tile([P, QW], F32, name="pst", tag="pst")
                    for h in range(2):
                        for oc in range(CC):
                            nc.tensor.matmul(pst[:, h * 512:(h + 1) * 512],
                                             Ks[oc][:, m * P:(m + 1) * P],
                                             Qs[oc][:, qp * QW + h * 512:qp * QW + (h + 1) * 512],
                                             start=(oc == 0), stop=(oc == CC - 1))
                    et = etp.tile([P, QW], ADT, name="et", tag="et")
                    nc.scalar.activation(out=et, in_=pst, func=AF.Exp)
                    for h in range(2):
                        for cc in range(CC):
                            nc.tensor.matmul(pav[cc][:, h * 512:(h + 1) * 512],
                                             VTs[:, m * C + cc * P: m * C + (cc + 1) * P],
                                             et[:, h * 512:(h + 1) * 512],
                                             start=(m == 0), stop=(m == NI - 1))
                    etv = et if ATTN_BF16 else et.bitcast(F32)
                    for h in range(2):
                        sl = slice(h * 512, (h + 1) * 512)
                        if m == 0:
                            acc_engs[h].tensor_copy(out=acc[:, sl], in_=etv[:, sl])
                        else:
                            acc_engs[h].tensor_tensor(out=acc[:, sl], in0=accf[:, sl],
                                                      in1=etv[:, sl], op=OP.add)
                # denominator -> broadcast reciprocal
                pd = psA.tile([P, QW], F32, name="psd", tag="pst")
                for h in range(2):
                    nc.tensor.matmul(pd[:, h * 512:(h + 1) * 512], allones_s,
                                     acc[:, h * 512:(h + 1) * 512], start=True, stop=True)
                rb = wkp.tile([P, QW], F32, name="rb", tag="rb")
                nc.vector.reciprocal_approx_fast(out=rb, in_=pd)
                obar = []
                for cc in range(CC):
                    ob = wkp.tile([P, QW], F32R, name="obar", tag="obar")
                    nc.vector.tensor_tensor(out=ob, in0=pav[cc], in1=rb, op=OP.mult)
                    obar.append(ob)
                for oc in range(CC):
                    py = psB.tile([P, QW], F32, name="psy", tag="pav")
                    for h in range(2):
                        for cc in range(CC):
                            nc.tensor.matmul(py[:, h * 512:(h + 1) * 512],
                                             wr[("wp", cc)][:, oc * P:(oc + 1) * P],
                                             obar[cc][:, h * 512:(h + 1) * 512],
                                             start=(cc == 0), stop=(cc == CC - 1))
                    y2 = wkp.tile([P, QW], F32, name="y2", tag="y2")
                    nc.vector.scalar_tensor_tensor(
                        out=y2, in0=py, scalar=beff[("wp", oc)],
                        in1=xs[oc].bitcast(F32)[:, qp * QW:(qp + 1) * QW],
                        op0=OP.add, op1=OP.add)
                    nc.sync.dma_start(
                        out=y_d[oc * P:(oc + 1) * P, qp * QW:(qp + 1) * QW], in_=y2)

    nc.finalize()
    return nc


def _get_nc():
    if "nc" not in _cache:
        _cache["nc"] = build_nc()
    return _cache["nc"]


def make_in_maps(x, gamma, beta, wq, bq, wk, bk, wv, bv, wp, bp):
    x = np.ascontiguousarray(np.asarray(x, dtype=np.float32))
    f32 = lambda a: np.ascontiguousarray(np.asarray(a, dtype=np.float32))
    scale = 1.0 / np.sqrt(np.float32(C))
    w_all = np.concatenate([
        f32(np.asarray(wq, np.float32).T),
        f32(np.asarray(wk, np.float32).T * scale),
        f32(np.asarray(wv, np.float32).T),
        f32(np.asarray(wp, np.float32).T)], axis=1)
    v_all = np.stack([
        f32(bq), f32(np.asarray(bk, np.float32) * scale), f32(bv), f32(bp),
        f32(gamma), f32(beta)], axis=1)
    shared = {"w_all": np.ascontiguousarray(w_all),
              "v_all": np.ascontiguousarray(v_all)}
    in_maps = []
    for core in range(NCORES):
        bi, half = core // 2, core % 2
        xb = x[bi].reshape(C, N)
        if half:
            # attention is permutation-equivariant over key tokens, so
            # rotating the columns lets every core treat columns 0:NQ as
            # its query half with an identical program.
            xb = np.roll(xb, -NQ, axis=1)
        m = dict(shared)
        m["x_r"] = round_tf32(xb)
        in_maps.append(m)
    return in_maps


def run(inputs: dict, trace: bool = False):
    from concourse.bass_utils import run_bass_kernel_spmd
    nc = _get_nc()
    in_maps = make_in_maps(**inputs)
    res = run_bass_kernel_spmd(nc, in_maps, core_ids=list(range(NCORES)), trace=trace)
    y = np.empty((B, C, N), dtype=np.float32)
    for core in range(NCORES):
        bi, half = core // 2, core % 2
        y[bi][:, half * NQ:(half + 1) * NQ] = res.results[core]["y"]
    return y.reshape(B, C, H, W), res


def kernel(**inputs) -> np.ndarray:
    out, _ = run(inputs, trace=False)
    return out
